# Initial kernel scaffold
#
"""Trainium2 Bass kernel for the CPG actor network (nn_Actor_CPG).

Strategy (pure data parallel over 8 NeuronCores, B rows split evenly):
- Host folds every tiny CPG matrix into one fused weight W [121, 108]:
  the device runs ONE fp16 matmul per 128-row chunk,
  out = XT_chunk.T @ W, where XT = [obs.T; r.T; th.T; rd.T; tdo.T;
  rddo.T; ones] is host-packed [121, B_shard]. The matmul emits, per
  row, all contraction quantities (Dd-term, sigma-term, Wv*lam_r,
  lam_th - Fiv) plus complete linear outputs (r_dot_dot and the
  trapezoidal-integration affine terms) directly in ROW-MAJOR PSUM.
- The remaining ~20 elementwise ops (sin/cos products, x/x_dot/x_ddot)
  run on VectorE/ScalarE/GpSimd over [128, 16, 12] tiles in fp32.
- All DRAM layouts are host-packed so every DMA moves >=512B per
  partition descriptor.
"""
import math

import numpy as np

B, N, P, PS, OBS = 524288, 12, 24, 12, 60
DT = 0.002
NCORES = 8
BSH = B // NCORES           # 65536 rows per core
CH = 128                    # rows per matmul chunk
PGC = 16                    # chunks per PSUM group
PGROWS = CH * PGC           # 2048
NPG = BSH // PGROWS         # 32
PG_PER_DG = 4               # psum groups per DMA group
DGROWS = PGROWS * PG_PER_DG  # 8192
NDG = BSH // DGROWS         # 8
IL = (BSH // CH) * N        # 6144 interleaved free dim
KX = 121                    # matmul contraction (60 obs + 5*12 state + 1)
NQ = 108                    # matmul output columns (9 quantities x 12)

NAT_NAMES = ["r_n", "th_n", "rd_n", "tdo_n", "rddo_n", "tddo_n"]

_cache = {}


def _split_waits_json(bir_bytes: bytes) -> bytes:
    """walrus in this image accepts ONE sync-wait per instruction; Tile
    emits several. Split them into single-wait NoOps (same engine,
    program order preserved)."""
    import json
    bir = json.loads(bir_bytes)
    for fn in bir.get("functions", []):
        for blk in fn.get("blocks", []):
            out = []
            for inst in blk.get("instructions", []):
                si = inst.get("sync_info")
                if isinstance(si, dict) and len(si.get("on_wait", [])) > 1:
                    waits = si["on_wait"]
                    for k, w in enumerate(waits[:-1]):
                        out.append({
                            "debug": inst.get("debug", 0),
                            "engine": inst["engine"],
                            "ins": [],
                            "name": f'{inst["name"]}-sw{k}',
                            "opcode": "NoOp",
                            "outs": [],
                            "sync_info": {"on_update": [], "on_wait": [w]},
                        })
                    si["on_wait"] = [waits[-1]]
                out.append(inst)
            blk["instructions"] = out
    return json.dumps(bir).encode()


def _install_birpatch():
    from concourse import bass2jax
    if getattr(bass2jax, "_ant_birpatch_installed", False):
        return
    orig = bass2jax._decompress_ant_bir

    def patched(ant_bir_value):
        return _split_waits_json(orig(ant_bir_value))

    bass2jax._decompress_ant_bir = patched
    bass2jax._ant_birpatch_installed = True


def _build_nc():
    from concourse import bass, mybir
    from concourse.tile import TileContext

    f32, f16 = mybir.dt.float32, mybir.dt.float16
    AF = mybir.ActivationFunctionType
    OP = mybir.AluOpType

    nc = bass.Bass()

    def reg_const(value, dtype=mybir.dt.float32):
        t = nc.alloc_sbuf_tensor(f"const-{dtype.name}-{value}", [128, 1], dtype)
        nc.gpsimd.memset(t.ap(), value)
        nc.const_aps.aps[(dtype, value)] = t.ap()

    reg_const(math.pi / 2)
    nc.all_engine_barrier()

    xt_d = nc.declare_dram_parameter("xt", [KX, BSH], f16, isOutput=False)
    wm_d = nc.declare_dram_parameter("wm", [KX, NQ], f16, isOutput=False)
    nat_d = {n: nc.declare_dram_parameter(n, [128, IL], f32, isOutput=False)
             for n in NAT_NAMES}
    out_d = nc.declare_dram_parameter("out", [9, 128, IL], f32, isOutput=True)

    with TileContext(nc) as tc:
        with tc.tile_pool(name="const", bufs=1) as cpool, \
             tc.tile_pool(name="xtp", bufs=3) as xtpool, \
             tc.tile_pool(name="natp", bufs=2) as natpool, \
             tc.tile_pool(name="outp", bufs=2) as outpool, \
             tc.tile_pool(name="midp", bufs=3) as midpool, \
             tc.tile_pool(name="psp", bufs=2, space="PSUM") as pspool:

            wm = cpool.tile([KX, NQ], f16, tag="wm")
            nc.sync.dma_start(out=wm[:, :], in_=wm_d[:, :])

            for dg in range(NDG):
                nat = {}
                for nname in NAT_NAMES:
                    t = natpool.tile([128, PG_PER_DG, PGC, N], f32, tag=nname)
                    nc.sync.dma_start(
                        out=t[:, :, :, :],
                        in_=nat_d[nname][:, dg * 768:(dg + 1) * 768]
                        .rearrange("p (s a b) -> p s a b", s=PG_PER_DG, a=PGC),
                    )
                    nat[nname] = t
                outs = [outpool.tile([128, PG_PER_DG, PGC, N], f32, tag=f"o{q}",
                                     name=f"o{q}")
                        for q in range(9)]

                for s in range(PG_PER_DG):
                    pg = dg * PG_PER_DG + s
                    xt = xtpool.tile([KX, PGROWS], f16, tag="xt", name="xt")
                    nc.sync.dma_start(
                        out=xt[:, :],
                        in_=xt_d[:, pg * PGROWS:(pg + 1) * PGROWS])
                    ps = pspool.tile([128, PGC, 128], f32, tag="ps", name="ps")
                    for c in range(PGC):
                        nc.tensor.matmul(
                            out=ps[:, c, 0:NQ],
                            lhsT=xt[:, c * CH:(c + 1) * CH],
                            rhs=wm[:, :],
                            start=True, stop=True)

                    def m(q):
                        return ps[:, :, q * N:(q + 1) * N]

                    def nv(nm):
                        return nat[nm][:, s, :, :]

                    def ov(q):
                        return outs[q][:, s, :, :]

                    def mid(nm):
                        t = midpool.tile([128, PGC, N], f32, tag=nm, name=nm)
                        return t[:, :, :]

                    cos_t, sin_t = mid("cos_t"), mid("sin_t")
                    sl, tdo2 = mid("sl"), mid("tdo2")
                    # ScalarE: transcendentals + psum evacuation copies
                    nc.scalar.activation(cos_t, nv("th_n"), AF.Sin,
                                         bias=math.pi / 2)
                    nc.scalar.activation(sin_t, nv("th_n"), AF.Sin)
                    nc.scalar.activation(sl, m(3), AF.Sin)
                    nc.scalar.activation(tdo2, nv("tdo_n"), AF.Square)
                    nc.scalar.activation(ov(8), m(4), AF.Copy)   # r_dot_dot
                    # theta_dot = m0 + m2*sl - m1*cos_t
                    p1, p2, t6 = mid("p1"), mid("p2"), mid("t6")
                    nc.vector.tensor_tensor(p1, m(2), sl, OP.mult)
                    nc.vector.tensor_tensor(p2, m(1), cos_t, OP.mult)
                    nc.vector.tensor_tensor(t6, m(0), p1, OP.add)
                    nc.vector.tensor_tensor(ov(4), t6, p2, OP.subtract)
                    # theta = m5 + theta_dot*DT/2 ; tdd = theta_dot/DT - tdo/DT
                    s1, s2 = mid("s1"), mid("s2")
                    nc.scalar.activation(s1, ov(4), AF.Copy, scale=DT / 2)
                    nc.scalar.activation(s2, ov(4), AF.Copy, scale=1.0 / DT)
                    nc.vector.tensor_tensor(ov(3), s1, m(5), OP.add)
                    nc.vector.tensor_tensor(ov(5), s2, m(6), OP.add)  # m6 = -tdo/DT
                    # r_dot = m7 + rdd*DT/2 ; r = m8 + rdd*DT^2/4
                    s3, s4 = mid("s3"), mid("s4")
                    nc.scalar.activation(s3, ov(8), AF.Copy, scale=DT / 2)
                    nc.scalar.activation(s4, ov(8), AF.Copy, scale=DT * DT / 4)
                    nc.vector.tensor_tensor(ov(7), s3, m(7), OP.add)
                    nc.vector.tensor_tensor(ov(6), s4, m(8), OP.add)
                    # x = r*cos ; x_dot = rd*cos - r*sin*tdo
                    st, rc, qq = mid("st"), mid("rc"), mid("qq")
                    nc.vector.tensor_tensor(ov(0), nv("r_n"), cos_t, OP.mult)
                    nc.vector.tensor_tensor(st, sin_t, nv("tdo_n"), OP.mult)
                    nc.vector.tensor_tensor(rc, nv("rd_n"), cos_t, OP.mult)
                    nc.vector.tensor_tensor(qq, nv("r_n"), st, OP.mult)
                    nc.vector.tensor_tensor(ov(1), rc, qq, OP.subtract)
                    # x_dd = cos*(rddo - r*tdo^2) - sin*(2*rd*tdo + r*tddo)
                    aa, bb, cc = mid("aa"), mid("bb"), mid("cc")
                    dd, ee, d2, ff, gg = (mid("dd"), mid("ee"), mid("d2"),
                                          mid("ff"), mid("gg"))
                    nc.gpsimd.tensor_tensor(aa, nv("r_n"), tdo2, OP.mult)
                    nc.gpsimd.tensor_tensor(bb, nv("rddo_n"), aa, OP.subtract)
                    nc.vector.tensor_tensor(cc, cos_t, bb, OP.mult)
                    nc.gpsimd.tensor_tensor(dd, nv("rd_n"), nv("tdo_n"), OP.mult)
                    nc.gpsimd.tensor_tensor(ee, nv("r_n"), nv("tddo_n"), OP.mult)
                    nc.gpsimd.tensor_scalar_mul(d2, dd, 2.0)
                    nc.gpsimd.tensor_tensor(ff, d2, ee, OP.add)
                    nc.vector.tensor_tensor(gg, sin_t, ff, OP.mult)
                    nc.vector.tensor_tensor(ov(2), cc, gg, OP.subtract)

                for q in range(9):
                    nc.sync.dma_start(
                        out=out_d[q, :, dg * 768:(dg + 1) * 768]
                        .rearrange("p (s a b) -> p s a b", s=PG_PER_DG, a=PGC),
                        in_=outs[q][:, :, :, :])
    return nc


def _fold_weights(inp):
    """Host-side constant folding -> W [121, 108] fp16 (fp64 math)."""
    g = {k: np.asarray(inp[k], np.float64) for k in
         ("v_short", "sym", "fixed", "Wd", "Ws", "Cd", "Od", "W", "Fi", "A",
          "Cr", "Or", "Lambda", "Lambda_T", "SIGMA", "D")}
    v = g["sym"] @ g["v_short"] + g["fixed"]
    Cdv, Odv = g["Cd"] @ v, g["Od"] @ v
    Wv, Fiv = g["W"] @ v, g["Fi"] @ v
    Av, Crv, Orv = g["A"] @ v, g["Cr"] @ v, g["Or"] @ v
    DWd = g["D"] @ g["Wd"]          # [12, 60]
    SWs = g["SIGMA"] @ g["Ws"]      # [12, 60]
    Lmd = g["Lambda"] - g["Lambda_T"]
    AvSq4 = Av * Av / 4.0
    a1, a0v = AvSq4 * Crv, AvSq4 * Orv

    W = np.zeros((KX, NQ), np.float64)
    two_pi = 2.0 * math.pi
    r0, rr, rth, rrd, rtdo, rrddo, rone = 0, 60, 72, 84, 96, 108, 120
    for n in range(N):
        # q0: m0 = 2pi*(Cdv*Dd + Odv)
        W[r0:r0 + 60, n] = two_pi * Cdv[n] * DWd[n]
        W[rone, n] = two_pi * Odv[n]
        # q1: m1 = sigma_N @ SIGMA.T
        W[r0:r0 + 60, 12 + n] = SWs[n]
        # q2: m2 = Wv * lam_r
        W[rr:rr + 12, 24 + n] = Wv[n] * g["Lambda"][n]
        # q3: m3 = lam_th - Fiv
        W[rth:rth + 12, 36 + n] = Lmd[n]
        W[rone, 36 + n] = -Fiv[n]
        # q4: rdd = a1*Dd + a0 - AvSq4*r - Av*rd
        W[r0:r0 + 60, 48 + n] = a1[n] * DWd[n]
        W[rr + n, 48 + n] = -AvSq4[n]
        W[rrd + n, 48 + n] = -Av[n]
        W[rone, 48 + n] = a0v[n]
        # q5: m5 = th + tdo*DT/2
        W[rth + n, 60 + n] = 1.0
        W[rtdo + n, 60 + n] = DT / 2
        # q6: m6 = -tdo/DT   (added, not subtracted, on device)
        W[rtdo + n, 72 + n] = -1.0 / DT
        # q7: m7 = rd + rddo*DT/2
        W[rrd + n, 84 + n] = 1.0
        W[rrddo + n, 84 + n] = DT / 2
        # q8: m8 = r + rd*DT + rddo*DT^2/4
        W[rr + n, 96 + n] = 1.0
        W[rrd + n, 96 + n] = DT
        W[rrddo + n, 96 + n] = DT * DT / 4
    return W.astype(np.float16)


def _interleave(arr):
    """[BSH, N] f32 -> [128, IL] so each partition holds its own rows."""
    return np.ascontiguousarray(
        arr.reshape(BSH // CH, CH, N).transpose(1, 0, 2).reshape(128, IL))


def kernel(**inputs):
    _install_birpatch()
    from concourse.bass_utils import run_bass_kernel_spmd

    inp = {k: np.asarray(v) for k, v in inputs.items()}
    Wm = _fold_weights(inp)

    obs = np.asarray(inp["obs"], np.float32)
    states = {k: np.asarray(inp[k], np.float32) for k in
              ("theta_old", "theta_dot_old", "theta_dot_dot_old",
               "r_old", "r_dot_old", "r_dot_dot_old")}

    in_maps = []
    for i in range(NCORES):
        sl = slice(i * BSH, (i + 1) * BSH)
        xt = np.empty((KX, BSH), np.float16)
        xt[0:60] = obs[sl].T
        xt[60:72] = states["r_old"][sl].T
        xt[72:84] = states["theta_old"][sl].T
        xt[84:96] = states["r_dot_old"][sl].T
        xt[96:108] = states["theta_dot_old"][sl].T
        xt[108:120] = states["r_dot_dot_old"][sl].T
        xt[120] = 1.0
        im = {
            "xt": xt,
            "wm": Wm,
            "r_n": _interleave(states["r_old"][sl]),
            "th_n": _interleave(states["theta_old"][sl]),
            "rd_n": _interleave(states["r_dot_old"][sl]),
            "tdo_n": _interleave(states["theta_dot_old"][sl]),
            "rddo_n": _interleave(states["r_dot_dot_old"][sl]),
            "tddo_n": _interleave(states["theta_dot_dot_old"][sl]),
        }
        in_maps.append(im)

    if "nc" not in _cache:
        _cache["nc"] = _build_nc()
    nc = _cache["nc"]

    trace = _cache.get("trace", False)
    res = run_bass_kernel_spmd(nc, in_maps, core_ids=list(range(NCORES)),
                               trace=trace)
    if trace:
        _cache["exec_time_ns"] = res.exec_time_ns
        _cache["profile_json"] = res.profile_json

    out = np.empty((9, B, N), np.float32)
    for i in range(NCORES):
        o = res.results[i]["out"]          # [9, 128, IL]
        o = o.reshape(9, 128, BSH // CH, N).transpose(0, 2, 1, 3)
        out[:, i * BSH:(i + 1) * BSH] = o.reshape(9, BSH, N)
    return out


# revision 6
# speedup vs baseline: 13.9096x; 13.9096x over previous
"""Trainium2 Bass kernel for the CPG actor network (nn_Actor_CPG).

Strategy (pure data parallel over 8 NeuronCores, B rows split evenly):
- Host folds every tiny CPG matrix into one fused weight W [121, 108]:
  the device runs ONE fp16 matmul per 128-row chunk,
  out = XT_chunk.T @ W, where XT = [obs.T; r.T; th.T; rd.T; tdo.T;
  rddo.T; ones] is host-packed [121, B_shard]. The matmul emits, per
  row, all contraction quantities (Dd-term, sigma-term, Wv*lam_r,
  lam_th - Fiv) plus complete linear outputs (r_dot_dot and the
  trapezoidal-integration affine terms) directly in ROW-MAJOR PSUM.
- The remaining ~20 elementwise ops (sin/cos products, x/x_dot/x_ddot)
  run on VectorE/ScalarE/GpSimd over [128, 16, 12] tiles in fp32.
- All DRAM layouts are host-packed so every DMA moves >=512B per
  partition descriptor.
"""
import math

import numpy as np

B, N, P, PS, OBS = 524288, 12, 24, 12, 60
DT = 0.002
NCORES = 8
BSH = B // NCORES           # 65536 rows per core
CH = 128                    # rows per matmul chunk
PGC = 16                    # chunks per PSUM group
PGROWS = CH * PGC           # 2048
NPG = BSH // PGROWS         # 32
PG_PER_DG = 4               # psum groups per DMA group
DGROWS = PGROWS * PG_PER_DG  # 8192
NDG = BSH // DGROWS         # 8
IL = (BSH // CH) * N        # 6144 interleaved free dim
KX = 121                    # matmul contraction (60 obs + 5*12 state + 1)
NQ = 108                    # matmul output columns (9 quantities x 12)

NAT_NAMES = ["r_n", "th_n", "rd_n", "tdo_n", "rddo_n", "tddo_n"]

_cache = {}


def _split_waits_json(bir_bytes: bytes) -> bytes:
    """walrus in this image accepts ONE sync-wait per instruction; Tile
    emits several. Split them into single-wait NoOps (same engine,
    program order preserved)."""
    import json
    bir = json.loads(bir_bytes)
    for fn in bir.get("functions", []):
        for blk in fn.get("blocks", []):
            out = []
            for inst in blk.get("instructions", []):
                si = inst.get("sync_info")
                if isinstance(si, dict) and len(si.get("on_wait", [])) > 1:
                    waits = si["on_wait"]
                    import os
                    carrier = os.environ.get("KCARRIER", "Drain")
                    for k, w in enumerate(waits[:-1]):
                        nop = {
                            "debug": inst.get("debug", 0),
                            "engine": inst["engine"],
                            "ins": [],
                            "name": f'{inst["name"]}-sw{k}',
                            "opcode": carrier,
                            "outs": [],
                            "sync_info": {"on_update": [], "on_wait": [w]},
                        }
                        if carrier == "Drain":
                            nop["is_reset_sema"] = False
                        out.append(nop)
                    si["on_wait"] = [waits[-1]]
                out.append(inst)
            blk["instructions"] = out
    return json.dumps(bir).encode()


def _install_birpatch():
    import sys
    import types
    # This image lacks antenv.axon_hooks (NTFF profiling); shim it so
    # run_bass_kernel_spmd's trace path degrades gracefully.
    if "antenv.axon_hooks" not in sys.modules:
        try:
            import antenv.axon_hooks  # noqa: F401
        except ImportError:
            mod = types.ModuleType("antenv.axon_hooks")
            mod.get_axon_ntff_profile_hook = lambda: None
            sys.modules["antenv.axon_hooks"] = mod
    from concourse import bass2jax
    if getattr(bass2jax, "_ant_birpatch_installed", False):
        return
    orig = bass2jax._decompress_ant_bir

    def patched(ant_bir_value):
        return _split_waits_json(orig(ant_bir_value))

    bass2jax._decompress_ant_bir = patched
    bass2jax._ant_birpatch_installed = True


def _build_nc(rep=1):
    from concourse import bass, mybir
    from concourse.tile import TileContext

    f32, f16 = mybir.dt.float32, mybir.dt.float16
    AF = mybir.ActivationFunctionType
    OP = mybir.AluOpType

    nc = bass.Bass()

    def reg_const(value, dtype=mybir.dt.float32):
        t = nc.alloc_sbuf_tensor(f"const-{dtype.name}-{value}", [128, 1], dtype)
        nc.gpsimd.memset(t.ap(), value)
        nc.const_aps.aps[(dtype, value)] = t.ap()

    reg_const(math.pi / 2)
    nc.all_engine_barrier()

    xt_d = nc.declare_dram_parameter("xt", [KX, BSH], f16, isOutput=False)
    wm_d = nc.declare_dram_parameter("wm", [KX, NQ], f16, isOutput=False)
    nat_d = {n: nc.declare_dram_parameter(n, [128, IL], f32, isOutput=False)
             for n in NAT_NAMES}
    out_d = nc.declare_dram_parameter("out", [9, 128, IL], f32, isOutput=True)

    with TileContext(nc) as tc:
        with tc.tile_pool(name="const", bufs=1) as cpool, \
             tc.tile_pool(name="xtp", bufs=3) as xtpool, \
             tc.tile_pool(name="natp", bufs=2) as natpool, \
             tc.tile_pool(name="outp", bufs=2) as outpool, \
             tc.tile_pool(name="midp", bufs=3) as midpool, \
             tc.tile_pool(name="psp", bufs=2, space="PSUM") as pspool:

            wm = cpool.tile([KX, NQ], f16, tag="wm")
            nc.sync.dma_start(out=wm[:, :], in_=wm_d[:, :])

            for dg in range(NDG * rep):
                dg = dg % NDG
                nat = {}
                for nname in NAT_NAMES:
                    t = natpool.tile([128, PG_PER_DG, PGC, N], f32, tag=nname)
                    nc.sync.dma_start(
                        out=t[:, :, :, :],
                        in_=nat_d[nname][:, dg * 768:(dg + 1) * 768]
                        .rearrange("p (s a b) -> p s a b", s=PG_PER_DG, a=PGC),
                    )
                    nat[nname] = t
                outs = [outpool.tile([128, PG_PER_DG, PGC, N], f32, tag=f"o{q}",
                                     name=f"o{q}")
                        for q in range(9)]

                for s in range(PG_PER_DG):
                    pg = dg * PG_PER_DG + s
                    xt = xtpool.tile([KX, PGROWS], f16, tag="xt", name="xt")
                    nc.sync.dma_start(
                        out=xt[:, :],
                        in_=xt_d[:, pg * PGROWS:(pg + 1) * PGROWS])
                    ps = pspool.tile([128, PGC, 128], f32, tag="ps", name="ps")
                    for c in range(PGC):
                        nc.tensor.matmul(
                            out=ps[:, c, 0:NQ],
                            lhsT=xt[:, c * CH:(c + 1) * CH],
                            rhs=wm[:, :],
                            start=True, stop=True)

                    def m(q):
                        return ps[:, :, q * N:(q + 1) * N]

                    def nv(nm):
                        return nat[nm][:, s, :, :]

                    def ov(q):
                        return outs[q][:, s, :, :]

                    def mid(nm):
                        t = midpool.tile([128, PGC, N], f32, tag=nm, name=nm)
                        return t[:, :, :]

                    cos_t, sin_t = mid("cos_t"), mid("sin_t")
                    sl, tdo2 = mid("sl"), mid("tdo2")
                    # ScalarE: transcendentals + psum evacuation copies
                    nc.scalar.activation(cos_t, nv("th_n"), AF.Sin,
                                         bias=math.pi / 2)
                    nc.scalar.activation(sin_t, nv("th_n"), AF.Sin)
                    nc.scalar.activation(sl, m(3), AF.Sin)
                    nc.scalar.activation(tdo2, nv("tdo_n"), AF.Square)
                    nc.scalar.activation(ov(8), m(4), AF.Copy)   # r_dot_dot
                    # theta_dot = m0 + m2*sl - m1*cos_t
                    p1, p2, t6 = mid("p1"), mid("p2"), mid("t6")
                    nc.vector.tensor_tensor(p1, m(2), sl, OP.mult)
                    nc.vector.tensor_tensor(p2, m(1), cos_t, OP.mult)
                    nc.vector.tensor_tensor(t6, m(0), p1, OP.add)
                    nc.vector.tensor_tensor(ov(4), t6, p2, OP.subtract)
                    # theta = m5 + theta_dot*DT/2 ; tdd = theta_dot/DT - tdo/DT
                    s1, s2 = mid("s1"), mid("s2")
                    nc.scalar.activation(s1, ov(4), AF.Copy, scale=DT / 2)
                    nc.scalar.activation(s2, ov(4), AF.Copy, scale=1.0 / DT)
                    nc.vector.tensor_tensor(ov(3), s1, m(5), OP.add)
                    nc.vector.tensor_tensor(ov(5), s2, m(6), OP.add)  # m6 = -tdo/DT
                    # r_dot = m7 + rdd*DT/2 ; r = m8 + rdd*DT^2/4
                    s3, s4 = mid("s3"), mid("s4")
                    nc.scalar.activation(s3, ov(8), AF.Copy, scale=DT / 2)
                    nc.scalar.activation(s4, ov(8), AF.Copy, scale=DT * DT / 4)
                    nc.vector.tensor_tensor(ov(7), s3, m(7), OP.add)
                    nc.vector.tensor_tensor(ov(6), s4, m(8), OP.add)
                    # x = r*cos ; x_dot = rd*cos - r*sin*tdo
                    st, rc, qq = mid("st"), mid("rc"), mid("qq")
                    nc.vector.tensor_tensor(ov(0), nv("r_n"), cos_t, OP.mult)
                    nc.vector.tensor_tensor(st, sin_t, nv("tdo_n"), OP.mult)
                    nc.vector.tensor_tensor(rc, nv("rd_n"), cos_t, OP.mult)
                    nc.vector.tensor_tensor(qq, nv("r_n"), st, OP.mult)
                    nc.vector.tensor_tensor(ov(1), rc, qq, OP.subtract)
                    # x_dd = cos*(rddo - r*tdo^2) - sin*(2*rd*tdo + r*tddo)
                    aa, bb, cc = mid("aa"), mid("bb"), mid("cc")
                    dd, ee, d2, ff, gg = (mid("dd"), mid("ee"), mid("d2"),
                                          mid("ff"), mid("gg"))
                    nc.gpsimd.tensor_tensor(aa, nv("r_n"), tdo2, OP.mult)
                    nc.gpsimd.tensor_tensor(bb, nv("rddo_n"), aa, OP.subtract)
                    nc.vector.tensor_tensor(cc, cos_t, bb, OP.mult)
                    nc.gpsimd.tensor_tensor(dd, nv("rd_n"), nv("tdo_n"), OP.mult)
                    nc.gpsimd.tensor_tensor(ee, nv("r_n"), nv("tddo_n"), OP.mult)
                    nc.gpsimd.tensor_scalar_mul(d2, dd, 2.0)
                    nc.gpsimd.tensor_tensor(ff, d2, ee, OP.add)
                    nc.vector.tensor_tensor(gg, sin_t, ff, OP.mult)
                    nc.vector.tensor_tensor(ov(2), cc, gg, OP.subtract)

                for q in range(9):
                    nc.sync.dma_start(
                        out=out_d[q, :, dg * 768:(dg + 1) * 768]
                        .rearrange("p (s a b) -> p s a b", s=PG_PER_DG, a=PGC),
                        in_=outs[q][:, :, :, :])
    return nc


def _fold_weights(inp):
    """Host-side constant folding -> W [121, 108] fp16 (fp64 math)."""
    g = {k: np.asarray(inp[k], np.float64) for k in
         ("v_short", "sym", "fixed", "Wd", "Ws", "Cd", "Od", "W", "Fi", "A",
          "Cr", "Or", "Lambda", "Lambda_T", "SIGMA", "D")}
    v = g["sym"] @ g["v_short"] + g["fixed"]
    Cdv, Odv = g["Cd"] @ v, g["Od"] @ v
    Wv, Fiv = g["W"] @ v, g["Fi"] @ v
    Av, Crv, Orv = g["A"] @ v, g["Cr"] @ v, g["Or"] @ v
    DWd = g["D"] @ g["Wd"]          # [12, 60]
    SWs = g["SIGMA"] @ g["Ws"]      # [12, 60]
    Lmd = g["Lambda"] - g["Lambda_T"]
    AvSq4 = Av * Av / 4.0
    a1, a0v = AvSq4 * Crv, AvSq4 * Orv

    W = np.zeros((KX, NQ), np.float64)
    two_pi = 2.0 * math.pi
    r0, rr, rth, rrd, rtdo, rrddo, rone = 0, 60, 72, 84, 96, 108, 120
    for n in range(N):
        # q0: m0 = 2pi*(Cdv*Dd + Odv)
        W[r0:r0 + 60, n] = two_pi * Cdv[n] * DWd[n]
        W[rone, n] = two_pi * Odv[n]
        # q1: m1 = sigma_N @ SIGMA.T
        W[r0:r0 + 60, 12 + n] = SWs[n]
        # q2: m2 = Wv * lam_r
        W[rr:rr + 12, 24 + n] = Wv[n] * g["Lambda"][n]
        # q3: m3 = lam_th - Fiv
        W[rth:rth + 12, 36 + n] = Lmd[n]
        W[rone, 36 + n] = -Fiv[n]
        # q4: rdd = a1*Dd + a0 - AvSq4*r - Av*rd
        W[r0:r0 + 60, 48 + n] = a1[n] * DWd[n]
        W[rr + n, 48 + n] = -AvSq4[n]
        W[rrd + n, 48 + n] = -Av[n]
        W[rone, 48 + n] = a0v[n]
        # q5: m5 = th + tdo*DT/2
        W[rth + n, 60 + n] = 1.0
        W[rtdo + n, 60 + n] = DT / 2
        # q6: m6 = -tdo/DT   (added, not subtracted, on device)
        W[rtdo + n, 72 + n] = -1.0 / DT
        # q7: m7 = rd + rddo*DT/2
        W[rrd + n, 84 + n] = 1.0
        W[rrddo + n, 84 + n] = DT / 2
        # q8: m8 = r + rd*DT + rddo*DT^2/4
        W[rr + n, 96 + n] = 1.0
        W[rrd + n, 96 + n] = DT
        W[rrddo + n, 96 + n] = DT * DT / 4
    return W.astype(np.float16)


def _interleave(arr):
    """[BSH, N] f32 -> [128, IL] so each partition holds its own rows."""
    return np.ascontiguousarray(
        arr.reshape(BSH // CH, CH, N).transpose(1, 0, 2).reshape(128, IL))


def _prepare_in_maps(inputs):
    inp = {k: np.asarray(v) for k, v in inputs.items()}
    Wm = _fold_weights(inp)

    obs = np.asarray(inp["obs"], np.float32)
    states = {k: np.asarray(inp[k], np.float32) for k in
              ("theta_old", "theta_dot_old", "theta_dot_dot_old",
               "r_old", "r_dot_old", "r_dot_dot_old")}

    in_maps = []
    for i in range(NCORES):
        sl = slice(i * BSH, (i + 1) * BSH)
        xt = np.empty((KX, BSH), np.float16)
        xt[0:60] = obs[sl].T
        xt[60:72] = states["r_old"][sl].T
        xt[72:84] = states["theta_old"][sl].T
        xt[84:96] = states["r_dot_old"][sl].T
        xt[96:108] = states["theta_dot_old"][sl].T
        xt[108:120] = states["r_dot_dot_old"][sl].T
        xt[120] = 1.0
        im = {
            "xt": xt,
            "wm": Wm,
            "r_n": _interleave(states["r_old"][sl]),
            "th_n": _interleave(states["theta_old"][sl]),
            "rd_n": _interleave(states["r_dot_old"][sl]),
            "tdo_n": _interleave(states["theta_dot_old"][sl]),
            "rddo_n": _interleave(states["r_dot_dot_old"][sl]),
            "tddo_n": _interleave(states["theta_dot_dot_old"][sl]),
        }
        in_maps.append(im)
    return in_maps


def kernel(**inputs):
    _install_birpatch()
    from concourse.bass_utils import run_bass_kernel_spmd

    in_maps = _prepare_in_maps(inputs)

    if "nc" not in _cache:
        _cache["nc"] = _build_nc()
    nc = _cache["nc"]

    trace = _cache.get("trace", False)
    res = run_bass_kernel_spmd(nc, in_maps, core_ids=list(range(NCORES)),
                               trace=trace)
    if trace:
        _cache["exec_time_ns"] = res.exec_time_ns
        _cache["profile_json"] = res.profile_json

    out = np.empty((9, B, N), np.float32)
    for i in range(NCORES):
        o = res.results[i]["out"]          # [9, 128, IL]
        o = o.reshape(9, 128, BSH // CH, N).transpose(0, 2, 1, 3)
        out[:, i * BSH:(i + 1) * BSH] = o.reshape(9, BSH, N)
    return out
